# revision 5
# baseline (speedup 1.0000x reference)
"""CQAttention (context-query attention) Trainium2 kernel.

Problem (per batch b of 16):
    S  = (C@w1)[:,None] + (Q@w2)[None,:] + (C*w3)@Q^T          [Lc, Lq]
    S1 = softmax_j(S masked by qmask), S2 = softmax_i(S masked by cmask)
    A  = S1@Q ;  Z = S2^T@C ;  Bm = S1@Z
    out = [C, A, C*A, C*Bm] @ out_w^T + out_b                  [Lc, d]
with B=16, Lc=1024, Lq=512, d=512, fp32.

Sharding: data-parallel over batch, 2 batches per NeuronCore, no
collectives.

Device mapping (single-S-pass design):
- Softmax shift-invariance: softmax_j(S) drops c1[i]=C@w1 and
  softmax_i(S) drops q2[j]=Q@w2, so the natural-layout exp uses the
  per-partition ACT bias c1[i]+cmask_bias[i] and the transposed-layout
  exp uses q2[j]+qmask_bias[j]. S_core = (C*w3)@Q^T is computed ONCE on
  the PE; the transposed copy comes from 32 PE transpose ops in fp16
  (1.0 c/row) instead of a full second matmul pass.
- c1, q2, and the w3 scaling of Q^T are host-side prep (rank-1 /
  diagonal terms, O(B*L*d) like the host transposes). All O(L^2*d) and
  O(L*d^2) matmuls run on device.
- Masked rows of e_c / e_qt are exactly 0 (exp(-1e4) == 0 in fp32), so
  Z and A need no separate masking and both softmax denominators are
  cheap N=2 column matmuls (the PE cost model charges by moving-dim
  size) sharing one PSUM bank; colsum rides the Z accumulation groups.
- 1/colsum is applied as a per-partition scale on Z's PSUM->SBUF copy
  (DVE); 1/rowsum columns are transposed back to a row with 8 tiny PE
  transposes, broadcast with two K=1 matmuls, and folded into the
  A^T/Bm^T PSUM->SBUF copies (DVE multiplies). No PE-stalling chains.
- dtypes: softmax/Z/A/B-path operands (c_nat, q_nat, e_c, e_qt, z) are
  fp16 (10-bit mantissa, ~f32r-class error, halves their DMA); the
  S/out-path operands (c_t, qw3_t, ow, at/cat/cbt staging) stay f32r --
  making those fp16 couples 3 extra quantizations into the final
  matmul and breaks the 2e-2 gate (measured 2.4e-2). fp32 accumulation
  in PSUM everywhere.
- Queue map (HWDGE triggers run on the issuing engine's SEQ, so the
  compute-free SP queue carries the big traffic): SP = ct/qw3t
  interleaved (ct split so the k-major opening groups start early),
  cn/qn, out_w in out-phase consumption order, final-batch stores;
  scalar = b0 bias columns; Pool SWDGE = small columns + non-final
  stores. 10 junk warmup matmuls spin the PE through its p-state ramp
  during the initial DMA fill; the final store is two N=256 groups so
  the tail ACT/DMA overlaps the last matmuls.
- split_multi_waits works around this container's walrus, which rejects
  any instruction carrying more than one sync wait.
"""

import numpy as np

import concourse.bass as bass
import concourse.mybir as mybir
import concourse.tile as tile
from concourse.bass_utils import run_bass_kernel_spmd
from concourse.masks import make_identity

F32 = mybir.dt.float32
F32R = mybir.dt.float32r
F16 = mybir.dt.float16
AF = mybir.ActivationFunctionType

B, LC, LQ, D = 16, 1024, 512, 512
NCORES = 8
BPC = B // NCORES  # batches per core
I_T, J_T, K_T = LC // 128, LQ // 128, D // 128  # 8, 4, 4
F_T = 4 * D // 128  # 16 feature tiles of out4
MASK_BIAS = 1.0e4  # exp(x - 1e4) == 0.0 exactly in fp32 for |x| ~ O(10)

SECTIONS = []


def _mark(nc, label):
    SECTIONS.append((label, int(nc.get_next_instruction_name().split("-")[1])))


def split_multi_waits(nc):
    """This walrus build allows at most one sync wait per instruction;
    hoist extras onto standalone EventSemaphore (wait) instructions."""
    for f in nc.m.functions:
        for blk in f.blocks:
            new = []
            changed = False
            for inst in blk.instructions:
                si = inst.sync_info
                waits = list(si.on_wait) if si is not None else []
                if len(waits) > 1:
                    changed = True
                    for k, w in enumerate(waits[:-1]):
                        ev = mybir.InstEventSemaphore(
                            name=f"{inst.name}-sw{k}", ins=[], outs=[]
                        )
                        ev.engine = inst.engine
                        ev.sync_info = mybir.SyncInfo(on_wait=[w], on_update=[])
                        new.append(ev)
                    si.on_wait = [waits[-1]]
                    inst.sync_info = si
                new.append(inst)
            if changed:
                blk.instructions = new


def _emit_batch(nc, tc, pools, consts, dram, b):
    (sb, small, psum, rowps) = pools
    (ones_r, ones_r2, ones_row, ident, ident_h, ow, obc) = consts

    _mark(nc, f"b{b}.in")
    # ---- input DMAs. Queue map (HWDGE triggers run on the issuing
    # engine's SEQ, so big inputs go via SP which has no compute):
    #   sync(SP):  ct+qw3t interleaved, then (b0 only) ow, b1 bias cols,
    #              and the final batch's out stores
    #   scalar:    b0 bias cols only (keeps the ACT SEQ clear)
    #   gpsimd:    cn, qn, non-final-batch out stores ----
    # fp16 C^T copy feeds the S matmuls (pairs with fp16 qw3t) so the
    # phase-1 critical wire is halved; the f32r C^T for cat/cbt/out
    # lands later when the wire has slack.
    ct16, qw3t = [], []
    for k in range(K_T):
        t = sb.tile([128, LC], F16, tag="ct16", bufs=4, name=f"ct16_{k}")
        nc.sync.dma_start(out=t[:], in_=dram["c_t16"].ap()[b, k * 128:(k + 1) * 128, :])
        ct16.append(t)
        t = sb.tile([128, LQ], F16, tag="qw3t", bufs=4, name=f"qw3t{k}")
        nc.sync.dma_start(out=t[:], in_=dram["qw3_t"].ap()[b, k * 128:(k + 1) * 128, :])
        qw3t.append(t)
    c1cb = small.tile([128, I_T], F32, tag="c1cb", bufs=2)
    nc.gpsimd.dma_start(out=c1cb[:], in_=dram["c1cb_col"].ap()[b])
    q2qb = small.tile([128, J_T], F32, tag="q2qb", bufs=2)
    nc.gpsimd.dma_start(out=q2qb[:], in_=dram["q2qb_col"].ap()[b])
    cn = []
    for i in range(I_T):
        t = sb.tile([128, D], F16, tag="cn", bufs=8, name=f"cn{i}")
        nc.sync.dma_start(out=t[:], in_=dram["c_nat"].ap()[b, i * 128:(i + 1) * 128, :])
        cn.append(t)
    qn = []
    for j in range(J_T):
        t = sb.tile([128, D], F16, tag="qn", bufs=4, name=f"qn{j}")
        nc.sync.dma_start(out=t[:], in_=dram["q_nat"].ap()[b, j * 128:(j + 1) * 128, :])
        qn.append(t)
    ct = []
    for k in range(K_T):
        t = sb.tile([128, LC], F32R, tag="ct", bufs=8, name=f"ct{k}")
        nc.sync.dma_start(out=t[:], in_=dram["c_t"].ap()[b, k * 128:(k + 1) * 128, :])
        ct.append(t)
    if b == 0:
        # out_w after b0's inputs in out-phase consumption order (k-major)
        # so the first out group is paced even if transfers run late
        for k in range(K_T):
            for g in range(4):
                f = g * 4 + k
                nc.sync.dma_start(out=ow[f][:],
                                  in_=dram["ow_t"].ap()[f * 128:(f + 1) * 128, :])

    # ---- phase 1: S_core = (C*w3)@Q^T; e_c = exp(.+c1+cb) (ACT);
    # s_sb copy (DVE); transposes of each half interleaved right after
    # its 4 S groups so eqt ACTs overlap the rest of phase 1 and the
    # st_ps PSUM tiles drain early ----
    _mark(nc, f"b{b}.s")
    e_c, s_sb = [], []
    eqt = [sb.tile([128, LC], F16, tag="eqt", bufs=4, name=f"eqt{j}")
           for j in range(J_T)]

    def s_group(i):
        s_ps = psum.tile([128, LQ], F32, tag="mmps", name=f"sps{i}")
        for k in range(K_T):
            nc.tensor.matmul(s_ps[:], ct16[k][:, i * 128:(i + 1) * 128], qw3t[k][:],
                             start=(k == 0), stop=(k == K_T - 1))
        ec = sb.tile([128, LQ], F16, tag="ec", bufs=8, name=f"ec{i}")
        nc.scalar.activation(ec[:], s_ps[:], AF.Exp,
                             bias=c1cb[:, i:i + 1], scale=1.0)
        ss = sb.tile([128, LQ], F16, tag="ssb", bufs=8, name=f"ssb{i}")
        nc.vector.tensor_copy(ss[:], s_ps[:])
        e_c.append(ec)
        s_sb.append(ss)

    def t_group(n, j):
        st_ps = psum.tile([128, 512], F16, tag="mmps", name=f"stps{n}_{j}")
        for ii in range(4):
            i = n * 4 + ii
            nc.tensor.transpose(st_ps[:, ii * 128:(ii + 1) * 128],
                                s_sb[i][:, j * 128:(j + 1) * 128], ident_h[:])
        nc.scalar.activation(eqt[j][:, n * 512:(n + 1) * 512], st_ps[:], AF.Exp,
                             bias=q2qb[:, j:j + 1], scale=1.0)

    if b == 0:
        # warm-up: junk matmuls during the input-DMA wait so the PE
        # p-state ramp completes before the first real matmul
        wu_ps = psum.tile([128, 128], F32, tag="mmps", name="wups")
        for w in range(4):
            nc.tensor.matmul(wu_ps[:], ident[:], ident[:], start=True, stop=True)
    # opening half k-major: consumes ct/qw3t tiles in DMA-arrival order
    # so the PE never idles between k's (and the p-state ramp completes)
    s_ps4 = [psum.tile([128, LQ], F32, tag="mmps", name=f"sps{i}")
             for i in range(4)]
    for k in range(K_T):
        for i in range(4):
            nc.tensor.matmul(s_ps4[i][:], ct16[k][:, i * 128:(i + 1) * 128],
                             qw3t[k][:], start=(k == 0), stop=(k == K_T - 1))
    for i in range(4):
        ec = sb.tile([128, LQ], F16, tag="ec", bufs=8, name=f"ec{i}")
        nc.scalar.activation(ec[:], s_ps4[i][:], AF.Exp,
                             bias=c1cb[:, i:i + 1], scale=1.0)
        ss = sb.tile([128, LQ], F16, tag="ssb", bufs=8, name=f"ssb{i}")
        nc.vector.tensor_copy(ss[:], s_ps4[i][:])
        e_c.append(ec)
        s_sb.append(ss)
    _mark(nc, f"b{b}.s2")
    for i in range(4, 8):
        s_group(i)
    _mark(nc, f"b{b}.tr0")
    for j in range(J_T):
        t_group(0, j)
    _mark(nc, f"b{b}.tr1")
    for j in range(J_T):
        t_group(1, j)

    # rowsums of E_qm (masked rows of eqt are exactly 0) as per-i-chunk
    # [128,2] columns: rs[i] = sum_j eqt[j][:, i]
    cs_ps = rowps.tile([128, 2 * J_T + 2 * I_T], F32, tag="csps", bufs=1)
    _mark(nc, f"b{b}.rsc")
    for ic in range(I_T):
        for j in range(J_T):
            nc.tensor.matmul(cs_ps[:, 8 + 2 * ic:8 + 2 * ic + 2],
                             eqt[j][:, ic * 128:(ic + 1) * 128], ones_r2[:],
                             start=(j == 0), stop=(j == J_T - 1))
    rs_sb = small.tile([128, 2 * I_T], F32R, tag="rs_sb", bufs=2)
    nc.scalar.copy(rs_sb[:], cs_ps[:, 8:8 + 2 * I_T])
    with nc.allow_low_precision(reason="f32r rounding of softmax denominators"):
        nc.vector.reciprocal(rs_sb[:], rs_sb[:])

    # ---- Z = S2^T @ C with colsum pairs riding in cols 0:8 of the
    # shared denominator PSUM bank ----
    rz_col = small.tile([128, J_T], F32, tag="rz", bufs=2)
    z = []

    def z_group(j):
        z_ps = psum.tile([128, D], F32, tag="mmps", name=f"zps{j}")
        for i in range(I_T):
            nc.tensor.matmul(z_ps[:], e_c[i][:, j * 128:(j + 1) * 128], cn[i][:],
                             start=(i == 0), stop=(i == I_T - 1))
            nc.tensor.matmul(cs_ps[:, 2 * j:2 * j + 2],
                             e_c[i][:, j * 128:(j + 1) * 128],
                             ones_r2[:], start=(i == 0), stop=(i == I_T - 1))
        nc.scalar.copy(rz_col[:, j:j + 1], cs_ps[:, 2 * j:2 * j + 1])
        nc.vector.reciprocal(rz_col[:, j:j + 1], rz_col[:, j:j + 1])
        zt = sb.tile([128, D], F16, tag="z", bufs=4, name=f"z{j}")
        nc.vector.tensor_scalar_mul(zt[:], z_ps[:], rz_col[:, j:j + 1])
        z.append(zt)

    _mark(nc, f"b{b}.z")
    z_group(0)
    z_group(1)
    z_group(2)
    # 1/rs columns -> row (8 tiny PE transposes) mid-Z so the ACT hops
    # in the chain hide under the remaining Z groups
    rs_row = small.tile([1, LC], F32R, tag="rs_row", bufs=2)
    irs = sb.tile([128, LC], F32, tag="irs", bufs=1)
    for n in range(2):
        irow_ps = psum.tile([1, 512], F32R, tag="mmps", name=f"irowps{n}")
        for ii in range(4):
            ic = n * 4 + ii
            nc.tensor.transpose(irow_ps[:, ii * 128:(ii + 1) * 128],
                                rs_sb[:, 2 * ic:2 * ic + 1], ident[:])
        nc.scalar.copy(rs_row[:, n * 512:(n + 1) * 512], irow_ps[:])
    z_group(3)
    _mark(nc, f"b{b}.irs")
    for n in range(2):
        irs_ps = psum.tile([128, 512], F32, tag="mmps", name=f"irsps{n}")
        nc.tensor.matmul(irs_ps[:], ones_row[:1, :128],
                         rs_row[:1, n * 512:(n + 1) * 512], start=True, stop=True)
        nc.scalar.copy(irs[:, n * 512:(n + 1) * 512], irs_ps[:])

    # ---- per n-chunk: A^T, Bm^T (1/rs folded into the PSUM->SBUF muls),
    # C*A, C*Bm staging, then the out matmuls ----
    for n in range(2):
        sl = slice(n * 512, (n + 1) * 512)
        _mark(nc, f"b{b}.ab{n}")
        at_n, cat_n, cbt_n = [], [], []
        for m in range(K_T):
            a_ps = psum.tile([128, 512], F32, tag="mmps", name=f"aps{n}_{m}")
            for j in range(J_T):
                nc.tensor.matmul(a_ps[:], qn[j][:, m * 128:(m + 1) * 128],
                                 eqt[j][:, sl],
                                 start=(j == 0), stop=(j == J_T - 1))
            at = sb.tile([128, 512], F32R, tag="at", bufs=4, name=f"at{m}_{n}")
            nc.vector.tensor_mul(at[:], a_ps[:], irs[:, sl])
            at_n.append(at)
            b_ps = psum.tile([128, 512], F32, tag="mmps", name=f"bps{n}_{m}")
            for j in range(J_T):
                nc.tensor.matmul(b_ps[:], z[j][:, m * 128:(m + 1) * 128],
                                 eqt[j][:, sl],
                                 start=(j == 0), stop=(j == J_T - 1))
            cbt = sb.tile([128, 512], F32R, tag="cbt", bufs=4, name=f"cbt{m}_{n}")
            nc.vector.tensor_mul(cbt[:], b_ps[:], irs[:, sl])
            cat = sb.tile([128, 512], F32R, tag="cat", bufs=4, name=f"cat{m}_{n}")
            nc.vector.tensor_mul(cat[:], ct[m][:, sl], at[:])
            cat_n.append(cat)
            nc.vector.tensor_mul(cbt[:], ct[m][:, sl], cbt[:])
            cbt_n.append(cbt)

        _mark(nc, f"b{b}.out{n}")
        # k-major accumulation order so each m-group consumes the DVE
        # products in the order the ab stage makes them; the final batch
        # stores via SP (free by then) so the tail isn't SWDGE-bound
        outq = nc.sync if b == BPC - 1 else nc.gpsimd
        def rhs_of(g, k, lo=0, hi=512):
            if g == 0:
                return ct[k][:, n * 512 + lo:n * 512 + hi]
            if g == 1:
                return at_n[k][:, lo:hi]
            if g == 2:
                return cat_n[k][:, lo:hi]
            return cbt_n[k][:, lo:hi]

        for m in range(K_T):
            if b == BPC - 1 and n == 1 and m == K_T - 1:
                # very last tile: two N=256 accumulation groups so the
                # first half's ACT+store overlap the second half's matmuls
                ot = sb.tile([128, 512], F32, tag="ot", bufs=4, name=f"ot{m}_{n}")
                for h in range(2):
                    hs = slice(h * 256, (h + 1) * 256)
                    o_ps = psum.tile([128, 256], F32, tag="mmps",
                                     name=f"ops{n}_{m}_{h}")
                    first = True
                    for k in range(K_T):
                        for g in range(4):
                            last = (k == K_T - 1 and g == 3)
                            nc.tensor.matmul(
                                o_ps[:], ow[g * 4 + k][:, m * 128:(m + 1) * 128],
                                rhs_of(g, k, h * 256, (h + 1) * 256),
                                start=first, stop=last)
                            first = False
                    nc.scalar.activation(ot[:, hs], o_ps[:], AF.Identity,
                                         bias=obc[:, m:m + 1], scale=1.0)
                    outq.dma_start(
                        out=dram["out_t"].ap()[b, m * 128:(m + 1) * 128,
                                               n * 512 + h * 256:
                                               n * 512 + (h + 1) * 256],
                        in_=ot[:, hs])
                continue
            o_ps = psum.tile([128, 512], F32, tag="mmps", name=f"ops{n}_{m}")
            first, last = True, False
            for k in range(K_T):
                for g in range(4):
                    last = (k == K_T - 1 and g == 3)
                    nc.tensor.matmul(o_ps[:], ow[g * 4 + k][:, m * 128:(m + 1) * 128],
                                     rhs_of(g, k), start=first, stop=last)
                    first = False
            ot = sb.tile([128, 512], F32, tag="ot", bufs=4, name=f"ot{m}_{n}")
            nc.scalar.activation(ot[:], o_ps[:], AF.Identity,
                                 bias=obc[:, m:m + 1], scale=1.0)
            outq.dma_start(
                out=dram["out_t"].ap()[b, m * 128:(m + 1) * 128, sl],
                in_=ot[:])


def build():
    nc = bass.Bass("TRN2", target_bir_lowering=False, debug=False,
                   num_devices=NCORES)
    dram = {}
    dram["c_t"] = nc.dram_tensor("c_t", [BPC, D, LC], F32R, kind="ExternalInput")
    dram["c_t16"] = nc.dram_tensor("c_t16", [BPC, D, LC], F16, kind="ExternalInput")
    dram["qw3_t"] = nc.dram_tensor("qw3_t", [BPC, D, LQ], F16, kind="ExternalInput")
    dram["c_nat"] = nc.dram_tensor("c_nat", [BPC, LC, D], F16, kind="ExternalInput")
    dram["q_nat"] = nc.dram_tensor("q_nat", [BPC, LQ, D], F16, kind="ExternalInput")
    dram["c1cb_col"] = nc.dram_tensor("c1cb_col", [BPC, 128, I_T], F32,
                                      kind="ExternalInput")
    dram["q2qb_col"] = nc.dram_tensor("q2qb_col", [BPC, 128, J_T], F32,
                                      kind="ExternalInput")
    dram["ow_t"] = nc.dram_tensor("ow_t", [4 * D, D], F32R, kind="ExternalInput")
    dram["ob_col"] = nc.dram_tensor("ob_col", [128, K_T], F32, kind="ExternalInput")
    dram["out_t"] = nc.dram_tensor("out_t", [BPC, D, LC], F32, kind="ExternalOutput")

    with tile.TileContext(nc) as tc:
        with tc.tile_pool(name="sb", bufs=4) as sb, \
             tc.tile_pool(name="small", bufs=1) as small, \
             tc.tile_pool(name="consts", bufs=1) as cpool, \
             tc.tile_pool(name="psum", bufs=7, space="PSUM") as psum, \
             tc.tile_pool(name="rowps", bufs=1, space="PSUM") as rowps:  # csps only
            ones_f = small.tile([128, 1], F32, tag="ones_f", bufs=1)
            nc.vector.memset(ones_f[:], 1.0)
            ones_r = cpool.tile([128, 1], F32R)
            nc.vector.tensor_copy(ones_r[:], ones_f[:])
            ones2_f = small.tile([128, 2], F32, tag="ones2_f", bufs=1)
            nc.vector.memset(ones2_f[:], 1.0)
            ones_r2 = cpool.tile([128, 2], F16)
            nc.vector.tensor_copy(ones_r2[:], ones2_f[:])
            onesrow_f = small.tile([1, 128], F32, tag="onesrow_f", bufs=1)
            nc.vector.memset(onesrow_f[:], 1.0)
            ones_row = cpool.tile([1, 128], F32R)
            nc.vector.tensor_copy(ones_row[:], onesrow_f[:])
            ident_f = small.tile([128, 128], F32, tag="ident_f", bufs=1)
            make_identity(nc, ident_f[:])
            ident = cpool.tile([128, 128], F32R)
            nc.vector.tensor_copy(ident[:], ident_f[:])
            ident_h = cpool.tile([128, 128], F16)
            nc.vector.tensor_copy(ident_h[:], ident_f[:])
            obc = cpool.tile([128, K_T], F32)
            nc.gpsimd.dma_start(out=obc[:], in_=dram["ob_col"].ap())
            ow = []
            for f in range(F_T):
                t = cpool.tile([128, D], F32R, tag="ow", bufs=F_T, name=f"ow{f}")
                ow.append(t)
            consts = (ones_r, ones_r2, ones_row, ident, ident_h, ow, obc)
            pools = (sb, small, psum, rowps)
            for b in range(BPC):
                _emit_batch(nc, tc, pools, consts, dram, b)

    split_multi_waits(nc)
    return nc


_NC = None


def _get_nc():
    global _NC
    if _NC is None:
        _NC = build()
    return _NC


def make_in_maps(C, Q, cmask, qmask, w, out_w, out_b):
    C = np.asarray(C, dtype=np.float32)
    Q = np.asarray(Q, dtype=np.float32)
    cmask = np.asarray(cmask, dtype=np.float32)
    qmask = np.asarray(qmask, dtype=np.float32)
    w = np.asarray(w, dtype=np.float32)
    out_w = np.asarray(out_w, dtype=np.float32)
    out_b = np.asarray(out_b, dtype=np.float32)

    w1, w2, w3 = w[:D], w[D:2 * D], w[2 * D:]
    c1cb = C @ w1 + (cmask - 1.0) * MASK_BIAS  # [B, LC]
    q2qb = Q @ w2 + (qmask - 1.0) * MASK_BIAS  # [B, LQ]
    qw3 = Q * w3  # [B, LQ, D]
    ow_t = np.ascontiguousarray(out_w.T)
    ob_col = np.ascontiguousarray(out_b.reshape(K_T, 128).T)

    in_maps = []
    for c in range(NCORES):
        sl = slice(c * BPC, (c + 1) * BPC)
        in_maps.append({
            "c_t": np.ascontiguousarray(C[sl].transpose(0, 2, 1)),
            "c_t16": np.ascontiguousarray(C[sl].transpose(0, 2, 1)).astype(np.float16),
            "qw3_t": np.ascontiguousarray(qw3[sl].transpose(0, 2, 1)).astype(np.float16),
            "c_nat": np.ascontiguousarray(C[sl]).astype(np.float16),
            "q_nat": np.ascontiguousarray(Q[sl]).astype(np.float16),
            "c1cb_col": np.ascontiguousarray(
                c1cb[sl].reshape(BPC, I_T, 128).transpose(0, 2, 1)),
            "q2qb_col": np.ascontiguousarray(
                q2qb[sl].reshape(BPC, J_T, 128).transpose(0, 2, 1)),
            "ow_t": ow_t, "ob_col": ob_col,
        })
    return in_maps


def kernel(C, Q, cmask, qmask, w, out_w, out_b):
    nc = _get_nc()
    in_maps = make_in_maps(C, Q, cmask, qmask, w, out_w, out_b)
    res = run_bass_kernel_spmd(nc, in_maps, list(range(NCORES)))
    outs = [res.results[i]["out_t"].transpose(0, 2, 1) for i in range(NCORES)]
    return np.ascontiguousarray(np.concatenate(outs, axis=0))
